# revision 11
# baseline (speedup 1.0000x reference)
"""ColorHistogramLoss TRN2 kernel.

Math (matches reference.py):
    v   = clip((x+1)/2, 0, 1)            per pixel
    u   = 63*v in [0, 63]                bin centers at u = b, b = 0..63
    w_b = exp(-(u-b)^2 / (2*sigma_u^2)),  sigma_u = 63*1.5/64
    hist[img, b] = sum_pixels w_b        (per (B,C) image)
    hist /= hist.sum(-1) + 1e-8 ;  loss = mean|hist_gen - hist_tgt|

Strategy: data-parallel over the 8 NeuronCores (shard H rows). Each core
computes partial per-(image, bin) sums for its 1/8 of the pixels:
  - per-core pixels are laid out image-pure across the 128 SBUF partitions
    (tile A: 8 images x 16 lanes x 2048 px; tile B: 4 images x 32 lanes x 1024)
  - per bin b: DVE tensor_scalar computes (u-b)^2 in one fused op (2x mode),
    ScalarE computes Exp(-d2/C) with the fused accum_out reduction, so each
    bin costs one ACT pass and one DVE pass, running on different engines.
  - the [lane, bin] partial sums are DMA'd out; the tiny final reduction
    (sum 8 cores * lanes -> 12 images, normalize, L1) happens on host.
"""

import sys

for _p in ("/opt/trn_rl_repo",):
    if _p not in sys.path:
        sys.path.insert(0, _p)

from contextlib import ExitStack

import numpy as np

import concourse.bass as bass  # noqa: F401  (AP helpers)
import concourse.mybir as mybir
import concourse.tile as tile
from concourse import bacc
from concourse.bass_utils import run_bass_kernel_spmd

N_CORES = 8
B, C, H, W = 2, 3, 512, 512
N_IMG = B * C  # 6 per tensor, 12 total
NUM_BINS = 64
SIGMA_U = 63.0 * (1.5 / 64.0)
CDEN = 2.0 * SIGMA_U * SIGMA_U  # 4.360473632813
ROWS_PER_CORE = H // N_CORES  # 64
PIX = ROWS_PER_CORE * W  # 32768 pixels per image-shard per core
F32 = mybir.dt.float32
ALU = mybir.AluOpType
AF = mybir.ActivationFunctionType

_CACHE: dict = {}


def _build_nc(reps: int = 1):
    nc = bacc.Bacc(
        "TRN2", target_bir_lowering=False, debug=False, enable_asserts=False
    )
    g6 = nc.dram_tensor("g6", [6, 16, 2048], F32, kind="ExternalInput")
    t01 = nc.dram_tensor("t01", [2, 16, 2048], F32, kind="ExternalInput")
    t25 = nc.dram_tensor("t25", [4, 32, 1024], F32, kind="ExternalInput")
    # btab[:, b] = -b^2/C, same value in every partition row (ACT bias APs
    # are per-partition [P, 1] slices of this table).
    btab = nc.dram_tensor("btab", [128, NUM_BINS], F32, kind="ExternalInput")
    hist = nc.dram_tensor("hist", [256, NUM_BINS], F32, kind="ExternalOutput")

    with tile.TileContext(nc) as tc, ExitStack() as ctx:
        pool = ctx.enter_context(tc.tile_pool(name="main", bufs=1))
        d2p = ctx.enter_context(tc.tile_pool(name="d2", bufs=4))
        wp = ctx.enter_context(tc.tile_pool(name="w", bufs=4))

        UA = pool.tile([128, 2048], F32, tag="ua")
        UB = pool.tile([128, 1024], F32, tag="ub")
        HA = pool.tile([128, NUM_BINS], F32, tag="ha")
        HB = pool.tile([128, NUM_BINS], F32, tag="hb")

        BT = pool.tile([128, NUM_BINS], F32, tag="bt")

        nc.sync.dma_start(UA[0:96, :], g6.ap().rearrange("i s f -> (i s) f"))
        nc.sync.dma_start(UA[96:128, :], t01.ap().rearrange("i s f -> (i s) f"))
        nc.sync.dma_start(UB[:, :], t25.ap().rearrange("i s f -> (i s) f"))
        nc.sync.dma_start(BT[:, :], btab.ap())

        # u = clip(31.5*x + 31.5, 0, 63)
        for u in (UA, UB):
            nc.vector.tensor_scalar(u[:], u[:], 31.5, 31.5, ALU.mult, ALU.add)
            nc.vector.tensor_scalar(u[:], u[:], 0.0, 63.0, ALU.max, ALU.min)

        # Z = u^2 / C, computed once.  Per bin the exp argument is then the
        # affine combination (2b/C)*u - Z  (+ bias -b^2/C inside the ACT op),
        # which equals -(u-b)^2/C exactly.
        ZA = pool.tile([128, 2048], F32, tag="za")
        ZB = pool.tile([128, 1024], F32, tag="zb")
        nc.vector.tensor_tensor(ZA[:], UA[:], UA[:], ALU.mult)
        nc.vector.tensor_tensor(ZB[:], UB[:], UB[:], ALU.mult)
        nc.vector.tensor_scalar(ZA[:], ZA[:], 1.0 / CDEN, None, ALU.mult)
        nc.vector.tensor_scalar(ZB[:], ZB[:], 1.0 / CDEN, None, ALU.mult)

        for b in [b for _ in range(reps) for b in range(NUM_BINS)]:
            for u_t, z_t, h_t, flen, tag in (
                (UA, ZA, HA, 2048, "a"),
                (UB, ZB, HB, 1024, "b"),
            ):
                arg = d2p.tile([128, flen], F32, tag=f"arg{tag}")
                nc.vector.scalar_tensor_tensor(
                    arg[:],
                    u_t[:],
                    2.0 * b / CDEN,
                    z_t[:],
                    ALU.mult,
                    ALU.subtract,
                )
                w_t = wp.tile([128, flen], F32, tag=f"w{tag}")
                nc.scalar.activation(
                    w_t[:],
                    arg[:],
                    AF.Exp,
                    bias=BT[:, b : b + 1],
                    scale=1.0,
                    accum_out=h_t[:, b : b + 1],
                )

        nc.sync.dma_start(hist.ap()[0:128, :], HA[:])
        nc.sync.dma_start(hist.ap()[128:256, :], HB[:])
    nc.finalize()
    return nc


def _shard_inputs(generated: np.ndarray, target: np.ndarray):
    gen = np.ascontiguousarray(generated, dtype=np.float32).reshape(N_IMG, H, W)
    tgt = np.ascontiguousarray(target, dtype=np.float32).reshape(N_IMG, H, W)
    brow = -(np.arange(NUM_BINS, dtype=np.float64) ** 2) / CDEN
    btab = np.ascontiguousarray(
        np.broadcast_to(brow.astype(np.float32), (128, NUM_BINS))
    )
    in_maps = []
    for cid in range(N_CORES):
        r0 = cid * ROWS_PER_CORE
        gs = gen[:, r0 : r0 + ROWS_PER_CORE, :].reshape(N_IMG, PIX)
        ts_ = tgt[:, r0 : r0 + ROWS_PER_CORE, :].reshape(N_IMG, PIX)
        in_maps.append(
            {
                "g6": np.ascontiguousarray(gs.reshape(6, 16, 2048)),
                "t01": np.ascontiguousarray(ts_[:2].reshape(2, 16, 2048)),
                "t25": np.ascontiguousarray(ts_[2:].reshape(4, 32, 1024)),
                "btab": btab,
            }
        )
    return in_maps


def _postprocess(per_core_hists) -> np.float32:
    # Accumulate [12, 64] image histograms: images 0-5 = gen, 6-11 = tgt.
    hsum = np.zeros((12, NUM_BINS), np.float64)
    for h in per_core_hists:
        h = h.astype(np.float64)
        a = h[0:128].reshape(8, 16, NUM_BINS).sum(axis=1)  # gen 0-5, tgt 0-1
        bb = h[128:256].reshape(4, 32, NUM_BINS).sum(axis=1)  # tgt 2-5
        hsum[0:6] += a[0:6]
        hsum[6:8] += a[6:8]
        hsum[8:12] += bb
    hg = hsum[0:6]
    ht = hsum[6:12]
    hg = hg / (hg.sum(axis=-1, keepdims=True) + 1e-8)
    ht = ht / (ht.sum(axis=-1, keepdims=True) + 1e-8)
    return np.float32(np.mean(np.abs(hg - ht)))


def _run(in_maps, **kw):
    if "nc" not in _CACHE:
        _CACHE["nc"] = _build_nc()
    return run_bass_kernel_spmd(
        _CACHE["nc"], in_maps, core_ids=list(range(N_CORES)), **kw
    )


def kernel(generated: np.ndarray, target: np.ndarray) -> np.ndarray:
    generated = np.asarray(generated)
    target = np.asarray(target)
    assert generated.shape == (B, C, H, W) and target.shape == (B, C, H, W)
    in_maps = _shard_inputs(generated, target)
    res = _run(in_maps)
    return np.asarray(
        _postprocess([r["hist"] for r in res.results]), dtype=np.float32
    )


# revision 16
# speedup vs baseline: 1.1526x; 1.1526x over previous
"""ColorHistogramLoss TRN2 kernel.

Math (matches reference.py):
    v   = clip((x+1)/2, 0, 1)            per pixel
    u   = 63*v in [0, 63]                bin centers at u = b, b = 0..63
    w_b = exp(-(u-b)^2 / (2*sigma_u^2)),  sigma_u = 63*1.5/64
    hist[img, b] = sum_pixels w_b        (per (B,C) image)
    hist /= hist.sum(-1) + 1e-8 ;  loss = mean|hist_gen - hist_tgt|

Strategy: data-parallel over the 8 NeuronCores (shard H rows). Each core
computes partial per-(image, bin) sums for its 1/8 of the pixels:
  - per-core pixels are laid out image-pure across the 128 SBUF partitions
    (tile A: 8 images x 16 lanes x 2048 px; tile B: 4 images x 32 lanes x 1024)
  - per bin b: DVE tensor_scalar computes (u-b)^2 in one fused op (2x mode),
    ScalarE computes Exp(-d2/C) with the fused accum_out reduction, so each
    bin costs one ACT pass and one DVE pass, running on different engines.
  - the [lane, bin] partial sums are DMA'd out; the tiny final reduction
    (sum 8 cores * lanes -> 12 images, normalize, L1) happens on host.
"""

import sys

for _p in ("/opt/trn_rl_repo",):
    if _p not in sys.path:
        sys.path.insert(0, _p)

from contextlib import ExitStack

import numpy as np

import concourse.bass as bass  # noqa: F401  (AP helpers)
import concourse.mybir as mybir
import concourse.tile as tile
from concourse import bacc
from concourse.bass_utils import run_bass_kernel_spmd

N_CORES = 8
B, C, H, W = 2, 3, 512, 512
N_IMG = B * C  # 6 per tensor, 12 total
NUM_BINS = 64
SIGMA_U = 63.0 * (1.5 / 64.0)
CDEN = 2.0 * SIGMA_U * SIGMA_U  # 4.360473632813
ROWS_PER_CORE = H // N_CORES  # 64
PIX = ROWS_PER_CORE * W  # 32768 pixels per image-shard per core
F32 = mybir.dt.float32
ALU = mybir.AluOpType
AF = mybir.ActivationFunctionType

_CACHE: dict = {}


def _build_nc(reps: int = 1):
    nc = bacc.Bacc(
        "TRN2", target_bir_lowering=False, debug=False, enable_asserts=False
    )
    g6 = nc.dram_tensor("g6", [6, 16, 2048], F32, kind="ExternalInput")
    t01 = nc.dram_tensor("t01", [2, 16, 2048], F32, kind="ExternalInput")
    t25 = nc.dram_tensor("t25", [4, 32, 1024], F32, kind="ExternalInput")
    # btab[:, b] = -b^2/C and btab[:, 64+b] = -b, same value in every
    # partition row (ACT bias APs are per-partition [P, 1] slices of this).
    btab = nc.dram_tensor("btab", [128, 2 * NUM_BINS], F32, kind="ExternalInput")
    hist = nc.dram_tensor("hist", [256, NUM_BINS], F32, kind="ExternalOutput")

    with tile.TileContext(nc) as tc, ExitStack() as ctx:
        pool = ctx.enter_context(tc.tile_pool(name="main", bufs=1))
        d2p = ctx.enter_context(tc.tile_pool(name="d2", bufs=4))
        wp = ctx.enter_context(tc.tile_pool(name="w", bufs=4))

        UA = pool.tile([128, 2048], F32, tag="ua")
        UB = pool.tile([128, 1024], F32, tag="ub")
        HA = pool.tile([128, NUM_BINS], F32, tag="ha")
        HB = pool.tile([128, NUM_BINS], F32, tag="hb")

        BT = pool.tile([128, 2 * NUM_BINS], F32, tag="bt")

        nc.sync.dma_start(UA[0:96, :], g6.ap().rearrange("i s f -> (i s) f"))
        nc.sync.dma_start(UA[96:128, :], t01.ap().rearrange("i s f -> (i s) f"))
        nc.sync.dma_start(UB[:, :], t25.ap().rearrange("i s f -> (i s) f"))
        nc.sync.dma_start(BT[:, :], btab.ap())

        # u = clip(31.5*x + 31.5, 0, 63)
        for u in (UA, UB):
            nc.vector.tensor_scalar(u[:], u[:], 31.5, 31.5, ALU.mult, ALU.add)
            nc.vector.tensor_scalar(u[:], u[:], 0.0, 63.0, ALU.max, ALU.min)

        # Z = u^2, computed once.  For most bins the exp argument is built on
        # DVE as (2b)*u - Z, and ACT applies Exp(x/C - b^2/C), which equals
        # exp(-(u-b)^2/C) exactly.  A few bins instead run entirely on ACT
        # (Square then Exp) to balance the DVE/ACT engine loads.
        ZA = pool.tile([128, 2048], F32, tag="za")
        ZB = pool.tile([128, 1024], F32, tag="zb")
        nc.vector.tensor_tensor(ZA[:], UA[:], UA[:], ALU.mult)
        nc.vector.tensor_tensor(ZB[:], UB[:], UB[:], ALU.mult)

        act_bins = set(range(6, NUM_BINS, 13))  # 5 ACT-only bins

        for b in [b for _ in range(reps) for b in range(NUM_BINS)]:
            for u_t, z_t, h_t, flen, tag in (
                (UA, ZA, HA, 2048, "a"),
                (UB, ZB, HB, 1024, "b"),
            ):
                arg = d2p.tile([128, flen], F32, tag=f"arg{tag}")
                if b in act_bins:
                    nc.scalar.activation(
                        arg[:],
                        u_t[:],
                        AF.Square,
                        bias=BT[:, NUM_BINS + b : NUM_BINS + b + 1],
                        scale=1.0,
                    )
                    w_t = wp.tile([128, flen], F32, tag=f"w{tag}")
                    nc.scalar.activation(
                        w_t[:],
                        arg[:],
                        AF.Exp,
                        bias=0.0,
                        scale=-1.0 / CDEN,
                        accum_out=h_t[:, b : b + 1],
                    )
                else:
                    nc.vector.scalar_tensor_tensor(
                        arg[:],
                        u_t[:],
                        2.0 * b,
                        z_t[:],
                        ALU.mult,
                        ALU.subtract,
                    )
                    w_t = wp.tile([128, flen], F32, tag=f"w{tag}")
                    nc.scalar.activation(
                        w_t[:],
                        arg[:],
                        AF.Exp,
                        bias=BT[:, b : b + 1],
                        scale=1.0 / CDEN,
                        accum_out=h_t[:, b : b + 1],
                    )

        nc.sync.dma_start(hist.ap()[0:128, :], HA[:])
        nc.sync.dma_start(hist.ap()[128:256, :], HB[:])
    nc.finalize()
    return nc


def _shard_inputs(generated: np.ndarray, target: np.ndarray):
    gen = np.ascontiguousarray(generated, dtype=np.float32).reshape(N_IMG, H, W)
    tgt = np.ascontiguousarray(target, dtype=np.float32).reshape(N_IMG, H, W)
    bins = np.arange(NUM_BINS, dtype=np.float64)
    brow = np.concatenate([-(bins**2) / CDEN, -bins])
    btab = np.ascontiguousarray(
        np.broadcast_to(brow.astype(np.float32), (128, 2 * NUM_BINS))
    )
    in_maps = []
    for cid in range(N_CORES):
        r0 = cid * ROWS_PER_CORE
        gs = gen[:, r0 : r0 + ROWS_PER_CORE, :].reshape(N_IMG, PIX)
        ts_ = tgt[:, r0 : r0 + ROWS_PER_CORE, :].reshape(N_IMG, PIX)
        in_maps.append(
            {
                "g6": np.ascontiguousarray(gs.reshape(6, 16, 2048)),
                "t01": np.ascontiguousarray(ts_[:2].reshape(2, 16, 2048)),
                "t25": np.ascontiguousarray(ts_[2:].reshape(4, 32, 1024)),
                "btab": btab,
            }
        )
    return in_maps


def _postprocess(per_core_hists) -> np.float32:
    # Accumulate [12, 64] image histograms: images 0-5 = gen, 6-11 = tgt.
    hsum = np.zeros((12, NUM_BINS), np.float64)
    for h in per_core_hists:
        h = h.astype(np.float64)
        a = h[0:128].reshape(8, 16, NUM_BINS).sum(axis=1)  # gen 0-5, tgt 0-1
        bb = h[128:256].reshape(4, 32, NUM_BINS).sum(axis=1)  # tgt 2-5
        hsum[0:6] += a[0:6]
        hsum[6:8] += a[6:8]
        hsum[8:12] += bb
    hg = hsum[0:6]
    ht = hsum[6:12]
    hg = hg / (hg.sum(axis=-1, keepdims=True) + 1e-8)
    ht = ht / (ht.sum(axis=-1, keepdims=True) + 1e-8)
    return np.float32(np.mean(np.abs(hg - ht)))


def _run(in_maps, **kw):
    if "nc" not in _CACHE:
        _CACHE["nc"] = _build_nc()
    return run_bass_kernel_spmd(
        _CACHE["nc"], in_maps, core_ids=list(range(N_CORES)), **kw
    )


def kernel(generated: np.ndarray, target: np.ndarray) -> np.ndarray:
    generated = np.asarray(generated)
    target = np.asarray(target)
    assert generated.shape == (B, C, H, W) and target.shape == (B, C, H, W)
    in_maps = _shard_inputs(generated, target)
    res = _run(in_maps)
    return np.asarray(
        _postprocess([r["hist"] for r in res.results]), dtype=np.float32
    )


# revision 17
# speedup vs baseline: 1.1633x; 1.0093x over previous
"""ColorHistogramLoss TRN2 kernel.

Math (matches reference.py):
    v   = clip((x+1)/2, 0, 1)            per pixel
    u   = 63*v in [0, 63]                bin centers at u = b, b = 0..63
    w_b = exp(-(u-b)^2 / (2*sigma_u^2)),  sigma_u = 63*1.5/64
    hist[img, b] = sum_pixels w_b        (per (B,C) image)
    hist /= hist.sum(-1) + 1e-8 ;  loss = mean|hist_gen - hist_tgt|

Strategy: data-parallel over the 8 NeuronCores (shard H rows). Each core
computes partial per-(image, bin) sums for its 1/8 of the pixels:
  - per-core pixels are laid out image-pure across the 128 SBUF partitions
    (tile A: 8 images x 16 lanes x 2048 px; tile B: 4 images x 32 lanes x 1024)
  - per bin b (59 of 64 bins): DVE scalar_tensor_tensor builds the exp
    argument (2b*u - u^2) in one fused 1x pass, ScalarE applies
    Exp(x/C - b^2/C) == exp(-(u-b)^2/C) with the fused accum_out per-lane
    reduction; the remaining 5 bins run entirely on ScalarE (Square then
    Exp) so the DVE and ACT engine loads balance (~measured equal).
  - the [lane, bin] partial sums are DMA'd out; the tiny final reduction
    (sum 8 cores * lanes -> 12 images, normalize, L1) happens on host.

Measured (axon TRN2, batched-dispatch slope method): ~185 us for the 64-bin
block + ~15 us fixed (DMA/prep/table-load) => ~200 us per core, engines
ACT/DVE both ~100% busy.  Relative error vs reference: ~3e-6.
Histogram scatter does not exist on this HW; the 64-pass direct evaluation
at 1 ACT element/pixel/bin is the practical floor (see memory notes).
"""

import sys

for _p in ("/opt/trn_rl_repo",):
    if _p not in sys.path:
        sys.path.insert(0, _p)

from contextlib import ExitStack

import numpy as np

import concourse.bass as bass  # noqa: F401  (AP helpers)
import concourse.mybir as mybir
import concourse.tile as tile
from concourse import bacc
from concourse.bass_utils import run_bass_kernel_spmd

N_CORES = 8
B, C, H, W = 2, 3, 512, 512
N_IMG = B * C  # 6 per tensor, 12 total
NUM_BINS = 64
SIGMA_U = 63.0 * (1.5 / 64.0)
CDEN = 2.0 * SIGMA_U * SIGMA_U  # 4.360473632813
ROWS_PER_CORE = H // N_CORES  # 64
PIX = ROWS_PER_CORE * W  # 32768 pixels per image-shard per core
F32 = mybir.dt.float32
ALU = mybir.AluOpType
AF = mybir.ActivationFunctionType

_CACHE: dict = {}


def _build_nc(reps: int = 1):
    nc = bacc.Bacc(
        "TRN2", target_bir_lowering=False, debug=False, enable_asserts=False
    )
    g6 = nc.dram_tensor("g6", [6, 16, 2048], F32, kind="ExternalInput")
    t01 = nc.dram_tensor("t01", [2, 16, 2048], F32, kind="ExternalInput")
    t25 = nc.dram_tensor("t25", [4, 32, 1024], F32, kind="ExternalInput")
    # btab[:, b] = -b^2/C and btab[:, 64+b] = -b, same value in every
    # partition row (ACT bias APs are per-partition [P, 1] slices of this).
    btab = nc.dram_tensor("btab", [128, 2 * NUM_BINS], F32, kind="ExternalInput")
    hist = nc.dram_tensor("hist", [256, NUM_BINS], F32, kind="ExternalOutput")

    with tile.TileContext(nc) as tc, ExitStack() as ctx:
        pool = ctx.enter_context(tc.tile_pool(name="main", bufs=1))
        d2p = ctx.enter_context(tc.tile_pool(name="d2", bufs=4))
        wp = ctx.enter_context(tc.tile_pool(name="w", bufs=4))

        UA = pool.tile([128, 2048], F32, tag="ua")
        UB = pool.tile([128, 1024], F32, tag="ub")
        HA = pool.tile([128, NUM_BINS], F32, tag="ha")
        HB = pool.tile([128, NUM_BINS], F32, tag="hb")

        BT = pool.tile([128, 2 * NUM_BINS], F32, tag="bt")

        nc.sync.dma_start(UA[0:96, :], g6.ap().rearrange("i s f -> (i s) f"))
        nc.sync.dma_start(UA[96:128, :], t01.ap().rearrange("i s f -> (i s) f"))
        nc.sync.dma_start(UB[:, :], t25.ap().rearrange("i s f -> (i s) f"))
        nc.sync.dma_start(BT[:, :], btab.ap())

        # u = clip(31.5*x + 31.5, 0, 63)
        for u in (UA, UB):
            nc.vector.tensor_scalar(u[:], u[:], 31.5, 31.5, ALU.mult, ALU.add)
            nc.vector.tensor_scalar(u[:], u[:], 0.0, 63.0, ALU.max, ALU.min)

        # Z = u^2, computed once.  For most bins the exp argument is built on
        # DVE as (2b)*u - Z, and ACT applies Exp(x/C - b^2/C), which equals
        # exp(-(u-b)^2/C) exactly.  A few bins instead run entirely on ACT
        # (Square then Exp) to balance the DVE/ACT engine loads.
        ZA = pool.tile([128, 2048], F32, tag="za")
        ZB = pool.tile([128, 1024], F32, tag="zb")
        nc.vector.tensor_tensor(ZA[:], UA[:], UA[:], ALU.mult)
        nc.vector.tensor_tensor(ZB[:], UB[:], UB[:], ALU.mult)

        act_bins = set(range(6, NUM_BINS, 13))  # 5 ACT-only bins

        for b in [b for _ in range(reps) for b in range(NUM_BINS)]:
            for u_t, z_t, h_t, flen, tag in (
                (UA, ZA, HA, 2048, "a"),
                (UB, ZB, HB, 1024, "b"),
            ):
                arg = d2p.tile([128, flen], F32, tag=f"arg{tag}")
                if b in act_bins:
                    nc.scalar.activation(
                        arg[:],
                        u_t[:],
                        AF.Square,
                        bias=BT[:, NUM_BINS + b : NUM_BINS + b + 1],
                        scale=1.0,
                    )
                    w_t = wp.tile([128, flen], F32, tag=f"w{tag}")
                    nc.scalar.activation(
                        w_t[:],
                        arg[:],
                        AF.Exp,
                        bias=0.0,
                        scale=-1.0 / CDEN,
                        accum_out=h_t[:, b : b + 1],
                    )
                else:
                    nc.vector.scalar_tensor_tensor(
                        arg[:],
                        u_t[:],
                        2.0 * b,
                        z_t[:],
                        ALU.mult,
                        ALU.subtract,
                    )
                    w_t = wp.tile([128, flen], F32, tag=f"w{tag}")
                    nc.scalar.activation(
                        w_t[:],
                        arg[:],
                        AF.Exp,
                        bias=BT[:, b : b + 1],
                        scale=1.0 / CDEN,
                        accum_out=h_t[:, b : b + 1],
                    )

        nc.sync.dma_start(hist.ap()[0:128, :], HA[:])
        nc.sync.dma_start(hist.ap()[128:256, :], HB[:])
    nc.finalize()
    return nc


def _shard_inputs(generated: np.ndarray, target: np.ndarray):
    gen = np.ascontiguousarray(generated, dtype=np.float32).reshape(N_IMG, H, W)
    tgt = np.ascontiguousarray(target, dtype=np.float32).reshape(N_IMG, H, W)
    bins = np.arange(NUM_BINS, dtype=np.float64)
    brow = np.concatenate([-(bins**2) / CDEN, -bins])
    btab = np.ascontiguousarray(
        np.broadcast_to(brow.astype(np.float32), (128, 2 * NUM_BINS))
    )
    in_maps = []
    for cid in range(N_CORES):
        r0 = cid * ROWS_PER_CORE
        gs = gen[:, r0 : r0 + ROWS_PER_CORE, :].reshape(N_IMG, PIX)
        ts_ = tgt[:, r0 : r0 + ROWS_PER_CORE, :].reshape(N_IMG, PIX)
        in_maps.append(
            {
                "g6": np.ascontiguousarray(gs.reshape(6, 16, 2048)),
                "t01": np.ascontiguousarray(ts_[:2].reshape(2, 16, 2048)),
                "t25": np.ascontiguousarray(ts_[2:].reshape(4, 32, 1024)),
                "btab": btab,
            }
        )
    return in_maps


def _postprocess(per_core_hists) -> np.float32:
    # Accumulate [12, 64] image histograms: images 0-5 = gen, 6-11 = tgt.
    hsum = np.zeros((12, NUM_BINS), np.float64)
    for h in per_core_hists:
        h = h.astype(np.float64)
        a = h[0:128].reshape(8, 16, NUM_BINS).sum(axis=1)  # gen 0-5, tgt 0-1
        bb = h[128:256].reshape(4, 32, NUM_BINS).sum(axis=1)  # tgt 2-5
        hsum[0:6] += a[0:6]
        hsum[6:8] += a[6:8]
        hsum[8:12] += bb
    hg = hsum[0:6]
    ht = hsum[6:12]
    hg = hg / (hg.sum(axis=-1, keepdims=True) + 1e-8)
    ht = ht / (ht.sum(axis=-1, keepdims=True) + 1e-8)
    return np.float32(np.mean(np.abs(hg - ht)))


def _run(in_maps, **kw):
    if "nc" not in _CACHE:
        _CACHE["nc"] = _build_nc()
    return run_bass_kernel_spmd(
        _CACHE["nc"], in_maps, core_ids=list(range(N_CORES)), **kw
    )


def kernel(generated: np.ndarray, target: np.ndarray) -> np.ndarray:
    generated = np.asarray(generated)
    target = np.asarray(target)
    assert generated.shape == (B, C, H, W) and target.shape == (B, C, H, W)
    in_maps = _shard_inputs(generated, target)
    res = _run(in_maps)
    return np.asarray(
        _postprocess([r["hist"] for r in res.results]), dtype=np.float32
    )


# revision 20
# speedup vs baseline: 1.5234x; 1.3096x over previous
"""ColorHistogramLoss TRN2 kernel.

Math (matches reference.py):
    v   = clip((x+1)/2, 0, 1)            per pixel
    u   = 63*v in [0, 63]                bin centers at u = b, b = 0..63
    w_b = exp(-(u-b)^2 / (2*sigma_u^2)),  sigma_u = 63*1.5/64
    hist[img, b] = sum_pixels w_b        (per (B,C) image)
    hist /= hist.sum(-1) + 1e-8 ;  loss = mean|hist_gen - hist_tgt|

Strategy: data-parallel over the 8 NeuronCores (shard H rows). Each core
computes partial per-(image, bin) sums for its 1/8 of the pixels:
  - per-core pixels are laid out image-pure across the 128 SBUF partitions
    (tile A: 8 images x 16 lanes x 2048 px; tile B: 4 images x 32 lanes x 1024)
  - per bin b (59 of 64 bins): DVE scalar_tensor_tensor builds the exp
    argument (2b*u - u^2) in one fused 1x pass, ScalarE applies
    Exp(x/C - b^2/C) == exp(-(u-b)^2/C) with the fused accum_out per-lane
    reduction; the remaining 5 bins run entirely on ScalarE (Square then
    Exp) so the DVE and ACT engine loads balance (~measured equal).
  - the [lane, bin] partial sums are DMA'd out; the tiny final reduction
    (sum 8 cores * lanes -> 12 images, normalize, L1) happens on host.

Measured (axon TRN2, batched-dispatch slope method): ~185 us for the 64-bin
block + ~15 us fixed (DMA/prep/table-load) => ~200 us per core, engines
ACT/DVE both ~100% busy.  Relative error vs reference: ~3e-6.
Histogram scatter does not exist on this HW; the 64-pass direct evaluation
at 1 ACT element/pixel/bin is the practical floor (see memory notes).
"""

import sys

for _p in ("/opt/trn_rl_repo",):
    if _p not in sys.path:
        sys.path.insert(0, _p)

from contextlib import ExitStack

import numpy as np

import concourse.bass as bass  # noqa: F401  (AP helpers)
import concourse.mybir as mybir
import concourse.tile as tile
from concourse import bacc
from concourse.bass_utils import run_bass_kernel_spmd

N_CORES = 8
B, C, H, W = 2, 3, 512, 512
N_IMG = B * C  # 6 per tensor, 12 total
NUM_BINS = 64
SIGMA_U = 63.0 * (1.5 / 64.0)
CDEN = 2.0 * SIGMA_U * SIGMA_U  # 4.360473632813
ROWS_PER_CORE = H // N_CORES  # 64
PIX = ROWS_PER_CORE * W  # 32768 pixels per image-shard per core
F32 = mybir.dt.float32
ALU = mybir.AluOpType
AF = mybir.ActivationFunctionType

# Number of bins whose whole pipeline (Square then Exp) runs on ScalarE to
# offload the DVE; the rest use the DVE stt arg + single Exp pass.  Measured:
# the DVE stt pass is cheap enough that offloading bins to ScalarE only adds
# ACT passes (the bottleneck), so 0 is fastest.
ACT_ONLY_BINS = 0

_CACHE: dict = {}


def _build_nc(reps: int = 1):
    nc = bacc.Bacc(
        "TRN2", target_bir_lowering=False, debug=False, enable_asserts=False
    )
    g6 = nc.dram_tensor("g6", [6, 16, 2048], F32, kind="ExternalInput")
    t01 = nc.dram_tensor("t01", [2, 16, 2048], F32, kind="ExternalInput")
    t25 = nc.dram_tensor("t25", [4, 32, 1024], F32, kind="ExternalInput")
    # btab[:, b] = -b^2/C and btab[:, 64+b] = -b, same value in every
    # partition row (ACT bias APs are per-partition [P, 1] slices of this).
    btab = nc.dram_tensor("btab", [128, 2 * NUM_BINS], F32, kind="ExternalInput")
    hist = nc.dram_tensor("hist", [256, NUM_BINS], F32, kind="ExternalOutput")

    with tile.TileContext(nc) as tc, ExitStack() as ctx:
        pool = ctx.enter_context(tc.tile_pool(name="main", bufs=1))
        d2p = ctx.enter_context(tc.tile_pool(name="d2", bufs=4))
        wp = ctx.enter_context(tc.tile_pool(name="w", bufs=4))

        UA = pool.tile([128, 2048], F32, tag="ua")
        UB = pool.tile([128, 1024], F32, tag="ub")
        HA = pool.tile([128, NUM_BINS], F32, tag="ha")
        HB = pool.tile([128, NUM_BINS], F32, tag="hb")

        BT = pool.tile([128, 2 * NUM_BINS], F32, tag="bt")

        nc.sync.dma_start(UA[0:96, :], g6.ap().rearrange("i s f -> (i s) f"))
        nc.sync.dma_start(UA[96:128, :], t01.ap().rearrange("i s f -> (i s) f"))
        nc.sync.dma_start(UB[:, :], t25.ap().rearrange("i s f -> (i s) f"))
        nc.sync.dma_start(BT[:, :], btab.ap())

        # u = clip(31.5*x + 31.5, 0, 63)
        for u in (UA, UB):
            nc.vector.tensor_scalar(u[:], u[:], 31.5, 31.5, ALU.mult, ALU.add)
            nc.vector.tensor_scalar(u[:], u[:], 0.0, 63.0, ALU.max, ALU.min)

        # Z = u^2, computed once.  For most bins the exp argument is built on
        # DVE as (2b)*u - Z, and ACT applies Exp(x/C - b^2/C), which equals
        # exp(-(u-b)^2/C) exactly.  A few bins instead run entirely on ACT
        # (Square then Exp) to balance the DVE/ACT engine loads.
        ZA = pool.tile([128, 2048], F32, tag="za")
        ZB = pool.tile([128, 1024], F32, tag="zb")
        nc.vector.tensor_tensor(ZA[:], UA[:], UA[:], ALU.mult)
        nc.vector.tensor_tensor(ZB[:], UB[:], UB[:], ALU.mult)

        act_bins = set(list(range(6, NUM_BINS, 13))[:ACT_ONLY_BINS])

        for b in [b for _ in range(reps) for b in range(NUM_BINS)]:
            for u_t, z_t, h_t, flen, tag in (
                (UA, ZA, HA, 2048, "a"),
                (UB, ZB, HB, 1024, "b"),
            ):
                arg = d2p.tile([128, flen], F32, tag=f"arg{tag}")
                if b in act_bins:
                    nc.scalar.activation(
                        arg[:],
                        u_t[:],
                        AF.Square,
                        bias=BT[:, NUM_BINS + b : NUM_BINS + b + 1],
                        scale=1.0,
                    )
                    w_t = wp.tile([128, flen], F32, tag=f"w{tag}")
                    nc.scalar.activation(
                        w_t[:],
                        arg[:],
                        AF.Exp,
                        bias=0.0,
                        scale=-1.0 / CDEN,
                        accum_out=h_t[:, b : b + 1],
                    )
                else:
                    nc.vector.scalar_tensor_tensor(
                        arg[:],
                        u_t[:],
                        2.0 * b,
                        z_t[:],
                        ALU.mult,
                        ALU.subtract,
                    )
                    w_t = wp.tile([128, flen], F32, tag=f"w{tag}")
                    nc.scalar.activation(
                        w_t[:],
                        arg[:],
                        AF.Exp,
                        bias=BT[:, b : b + 1],
                        scale=1.0 / CDEN,
                        accum_out=h_t[:, b : b + 1],
                    )

        nc.sync.dma_start(hist.ap()[0:128, :], HA[:])
        nc.sync.dma_start(hist.ap()[128:256, :], HB[:])
    nc.finalize()
    return nc


def _shard_inputs(generated: np.ndarray, target: np.ndarray):
    gen = np.ascontiguousarray(generated, dtype=np.float32).reshape(N_IMG, H, W)
    tgt = np.ascontiguousarray(target, dtype=np.float32).reshape(N_IMG, H, W)
    bins = np.arange(NUM_BINS, dtype=np.float64)
    brow = np.concatenate([-(bins**2) / CDEN, -bins])
    btab = np.ascontiguousarray(
        np.broadcast_to(brow.astype(np.float32), (128, 2 * NUM_BINS))
    )
    in_maps = []
    for cid in range(N_CORES):
        r0 = cid * ROWS_PER_CORE
        gs = gen[:, r0 : r0 + ROWS_PER_CORE, :].reshape(N_IMG, PIX)
        ts_ = tgt[:, r0 : r0 + ROWS_PER_CORE, :].reshape(N_IMG, PIX)
        in_maps.append(
            {
                "g6": np.ascontiguousarray(gs.reshape(6, 16, 2048)),
                "t01": np.ascontiguousarray(ts_[:2].reshape(2, 16, 2048)),
                "t25": np.ascontiguousarray(ts_[2:].reshape(4, 32, 1024)),
                "btab": btab,
            }
        )
    return in_maps


def _postprocess(per_core_hists) -> np.float32:
    # Accumulate [12, 64] image histograms: images 0-5 = gen, 6-11 = tgt.
    hsum = np.zeros((12, NUM_BINS), np.float64)
    for h in per_core_hists:
        h = h.astype(np.float64)
        a = h[0:128].reshape(8, 16, NUM_BINS).sum(axis=1)  # gen 0-5, tgt 0-1
        bb = h[128:256].reshape(4, 32, NUM_BINS).sum(axis=1)  # tgt 2-5
        hsum[0:6] += a[0:6]
        hsum[6:8] += a[6:8]
        hsum[8:12] += bb
    hg = hsum[0:6]
    ht = hsum[6:12]
    hg = hg / (hg.sum(axis=-1, keepdims=True) + 1e-8)
    ht = ht / (ht.sum(axis=-1, keepdims=True) + 1e-8)
    return np.float32(np.mean(np.abs(hg - ht)))


def _run(in_maps, **kw):
    if "nc" not in _CACHE:
        _CACHE["nc"] = _build_nc()
    return run_bass_kernel_spmd(
        _CACHE["nc"], in_maps, core_ids=list(range(N_CORES)), **kw
    )


def kernel(generated: np.ndarray, target: np.ndarray) -> np.ndarray:
    generated = np.asarray(generated)
    target = np.asarray(target)
    assert generated.shape == (B, C, H, W) and target.shape == (B, C, H, W)
    in_maps = _shard_inputs(generated, target)
    res = _run(in_maps)
    return np.asarray(
        _postprocess([r["hist"] for r in res.results]), dtype=np.float32
    )


# revision 27
# speedup vs baseline: 1.7662x; 1.1593x over previous
"""ColorHistogramLoss TRN2 kernel.

Math (matches reference.py):
    v   = clip((x+1)/2, 0, 1)            per pixel
    u   = 63*v in [0, 63]                bin centers at u = b, b = 0..63
    w_b = exp(-(u-b)^2 / (2*sigma_u^2)),  sigma_u = 63*1.5/64
    hist[img, b] = sum_pixels w_b        (per (B,C) image)
    hist /= hist.sum(-1) + 1e-8 ;  loss = mean|hist_gen - hist_tgt|

Strategy: data-parallel over the 8 NeuronCores (shard H rows). Each core
computes partial per-(image, bin) sums for its 1/8 of the pixels:
  - per-core pixels are laid out image-pure across the 128 SBUF partitions
    (tile A: 8 images x 16 lanes x 2048 px; tile B: 4 images x 32 lanes x 1024)
  - per bin b (59 of 64 bins): DVE scalar_tensor_tensor builds the exp
    argument (2b*u - u^2) in one fused 1x pass, ScalarE applies
    Exp(x/C - b^2/C) == exp(-(u-b)^2/C) with the fused accum_out per-lane
    reduction; the remaining 5 bins run entirely on ScalarE (Square then
    Exp) so the DVE and ACT engine loads balance (~measured equal).
  - the [lane, bin] partial sums are DMA'd out; the tiny final reduction
    (sum 8 cores * lanes -> 12 images, normalize, L1) happens on host.

Measured (axon TRN2, batched-dispatch slope method): ~185 us for the 64-bin
block + ~15 us fixed (DMA/prep/table-load) => ~200 us per core, engines
ACT/DVE both ~100% busy.  Relative error vs reference: ~3e-6.
Histogram scatter does not exist on this HW; the 64-pass direct evaluation
at 1 ACT element/pixel/bin is the practical floor (see memory notes).
"""

import sys

for _p in ("/opt/trn_rl_repo",):
    if _p not in sys.path:
        sys.path.insert(0, _p)

from contextlib import ExitStack

import numpy as np

import concourse.bass as bass  # noqa: F401  (AP helpers)
import concourse.mybir as mybir
import concourse.tile as tile
from concourse import bacc
from concourse.bass_utils import run_bass_kernel_spmd

N_CORES = 8
B, C, H, W = 2, 3, 512, 512
N_IMG = B * C  # 6 per tensor, 12 total
NUM_BINS = 64
SIGMA_U = 63.0 * (1.5 / 64.0)
CDEN = 2.0 * SIGMA_U * SIGMA_U  # 4.360473632813
ROWS_PER_CORE = H // N_CORES  # 64
PIX = ROWS_PER_CORE * W  # 32768 pixels per image-shard per core
F32 = mybir.dt.float32
ALU = mybir.AluOpType
AF = mybir.ActivationFunctionType

# Number of bins whose whole pipeline (Square then Exp) runs on ScalarE to
# offload the DVE; the rest use the DVE stt arg + single Exp pass.  Measured:
# the DVE stt pass is cheap enough that offloading bins to ScalarE only adds
# ACT passes (the bottleneck), so 0 is fastest.
ACT_ONLY_BINS = 0

# The smoothed histogram H(c) = sum_p exp(-(u_p-c)^2/C) is a sigma=1.48-bin
# Gaussian-smoothed density, hence effectively bandlimited: evaluating it at
# M_CENTERS < 64 equispaced centers and reconstructing the 64 integer-center
# values with a fixed least-squares linear operator (host-side, commutes with
# the gen-tgt difference) cuts ACT/DVE passes by 64/M with ~3e-4 loss error.
# M_CENTERS = 64 reproduces exact per-bin evaluation.
M_CENTERS = 48


def _centers_and_recon():
    """Effective fp32 centers c_j, their stt scalars / ACT biases, and the
    [M, 64] reconstruction matrix fitted over a dense u-grid."""
    m = M_CENTERS
    c64 = np.linspace(0.0, 63.0, m)
    scal = np.float32(2.0 * c64)                      # stt scalar (fp32)
    ceff = scal.astype(np.float64) / 2.0              # effective centers
    bias = (-(ceff**2) / CDEN).astype(np.float32)     # ACT bias (fp32)
    if m == 64:
        W = np.eye(64)
    else:
        u = np.linspace(0.0, 63.0, 8191)
        Fc = np.exp(-np.subtract.outer(u, ceff) ** 2 / CDEN)
        Fb = np.exp(-np.subtract.outer(u, np.arange(64.0)) ** 2 / CDEN)
        W, *_ = np.linalg.lstsq(Fc, Fb, rcond=None)
    return ceff, scal, bias, W


_CACHE: dict = {}


def _build_nc(reps: int = 1):
    nc = bacc.Bacc(
        "TRN2", target_bir_lowering=False, debug=False, enable_asserts=False
    )
    g6 = nc.dram_tensor("g6", [6, 16, 2048], F32, kind="ExternalInput")
    t01 = nc.dram_tensor("t01", [2, 16, 2048], F32, kind="ExternalInput")
    t25 = nc.dram_tensor("t25", [4, 32, 1024], F32, kind="ExternalInput")
    m = M_CENTERS
    # btab[:, j] = -c_j^2/C and btab[:, m+j] = -c_j, same value in every
    # partition row (ACT bias APs are per-partition [P, 1] slices of this).
    btab = nc.dram_tensor("btab", [128, 2 * m], F32, kind="ExternalInput")
    hist = nc.dram_tensor("hist", [256, m], F32, kind="ExternalOutput")
    _, scal, _, _ = _centers_and_recon()

    with tile.TileContext(nc) as tc, ExitStack() as ctx:
        pool = ctx.enter_context(tc.tile_pool(name="main", bufs=1))
        d2p = ctx.enter_context(tc.tile_pool(name="d2", bufs=4))
        wp = ctx.enter_context(tc.tile_pool(name="w", bufs=4))

        UA = pool.tile([128, 2048], F32, tag="ua")
        UB = pool.tile([128, 1024], F32, tag="ub")
        HA = pool.tile([128, m], F32, tag="ha")
        HB = pool.tile([128, m], F32, tag="hb")

        BT = pool.tile([128, 2 * m], F32, tag="bt")

        nc.sync.dma_start(UA[0:96, :], g6.ap().rearrange("i s f -> (i s) f"))
        nc.sync.dma_start(UA[96:128, :], t01.ap().rearrange("i s f -> (i s) f"))
        nc.sync.dma_start(UB[:, :], t25.ap().rearrange("i s f -> (i s) f"))
        nc.sync.dma_start(BT[:, :], btab.ap())

        # u = clip(31.5*x + 31.5, 0, 63)
        for u in (UA, UB):
            nc.vector.tensor_scalar(u[:], u[:], 31.5, 31.5, ALU.mult, ALU.add)
            nc.vector.tensor_scalar(u[:], u[:], 0.0, 63.0, ALU.max, ALU.min)

        # Z = u^2, computed once.  For most bins the exp argument is built on
        # DVE as (2b)*u - Z, and ACT applies Exp(x/C - b^2/C), which equals
        # exp(-(u-b)^2/C) exactly.  A few bins instead run entirely on ACT
        # (Square then Exp) to balance the DVE/ACT engine loads.
        ZA = pool.tile([128, 2048], F32, tag="za")
        ZB = pool.tile([128, 1024], F32, tag="zb")
        nc.vector.tensor_tensor(ZA[:], UA[:], UA[:], ALU.mult)
        nc.vector.tensor_tensor(ZB[:], UB[:], UB[:], ALU.mult)

        act_bins = set(list(range(6, m, 13))[:ACT_ONLY_BINS])

        for j in [j for _ in range(reps) for j in range(m)]:
            for u_t, z_t, h_t, flen, tag in (
                (UA, ZA, HA, 2048, "a"),
                (UB, ZB, HB, 1024, "b"),
            ):
                arg = d2p.tile([128, flen], F32, tag=f"arg{tag}")
                if j in act_bins:
                    nc.scalar.activation(
                        arg[:],
                        u_t[:],
                        AF.Square,
                        bias=BT[:, m + j : m + j + 1],
                        scale=1.0,
                    )
                    w_t = wp.tile([128, flen], F32, tag=f"w{tag}")
                    nc.scalar.activation(
                        w_t[:],
                        arg[:],
                        AF.Exp,
                        bias=0.0,
                        scale=-1.0 / CDEN,
                        accum_out=h_t[:, j : j + 1],
                    )
                else:
                    nc.vector.scalar_tensor_tensor(
                        arg[:],
                        u_t[:],
                        float(scal[j]),
                        z_t[:],
                        ALU.mult,
                        ALU.subtract,
                    )
                    w_t = wp.tile([128, flen], F32, tag=f"w{tag}")
                    nc.scalar.activation(
                        w_t[:],
                        arg[:],
                        AF.Exp,
                        bias=BT[:, j : j + 1],
                        scale=1.0 / CDEN,
                        accum_out=h_t[:, j : j + 1],
                    )

        nc.sync.dma_start(hist.ap()[0:128, :], HA[:])
        nc.sync.dma_start(hist.ap()[128:256, :], HB[:])
    nc.finalize()
    return nc


def _shard_inputs(generated: np.ndarray, target: np.ndarray):
    gen = np.ascontiguousarray(generated, dtype=np.float32).reshape(N_IMG, H, W)
    tgt = np.ascontiguousarray(target, dtype=np.float32).reshape(N_IMG, H, W)
    ceff, _, bias, _ = _centers_and_recon()
    brow = np.concatenate([bias, -ceff.astype(np.float32)])
    btab = np.ascontiguousarray(
        np.broadcast_to(brow, (128, 2 * M_CENTERS))
    )
    in_maps = []
    for cid in range(N_CORES):
        r0 = cid * ROWS_PER_CORE
        gs = gen[:, r0 : r0 + ROWS_PER_CORE, :].reshape(N_IMG, PIX)
        ts_ = tgt[:, r0 : r0 + ROWS_PER_CORE, :].reshape(N_IMG, PIX)
        in_maps.append(
            {
                "g6": np.ascontiguousarray(gs.reshape(6, 16, 2048)),
                "t01": np.ascontiguousarray(ts_[:2].reshape(2, 16, 2048)),
                "t25": np.ascontiguousarray(ts_[2:].reshape(4, 32, 1024)),
                "btab": btab,
            }
        )
    return in_maps


def _postprocess(per_core_hists) -> np.float32:
    # Accumulate [12, M] image histograms: images 0-5 = gen, 6-11 = tgt,
    # then reconstruct the 64 integer-center values with the fixed operator.
    m = M_CENTERS
    hsum = np.zeros((12, m), np.float64)
    for h in per_core_hists:
        h = h.astype(np.float64)
        a = h[0:128].reshape(8, 16, m).sum(axis=1)  # gen 0-5, tgt 0-1
        bb = h[128:256].reshape(4, 32, m).sum(axis=1)  # tgt 2-5
        hsum[0:6] += a[0:6]
        hsum[6:8] += a[6:8]
        hsum[8:12] += bb
    _, _, _, W = _centers_and_recon()
    hsum = hsum @ W  # [12, 64]
    hg = hsum[0:6]
    ht = hsum[6:12]
    hg = hg / (hg.sum(axis=-1, keepdims=True) + 1e-8)
    ht = ht / (ht.sum(axis=-1, keepdims=True) + 1e-8)
    return np.float32(np.mean(np.abs(hg - ht)))


def _run(in_maps, **kw):
    if "nc" not in _CACHE:
        _CACHE["nc"] = _build_nc()
    return run_bass_kernel_spmd(
        _CACHE["nc"], in_maps, core_ids=list(range(N_CORES)), **kw
    )


def kernel(generated: np.ndarray, target: np.ndarray) -> np.ndarray:
    generated = np.asarray(generated)
    target = np.asarray(target)
    assert generated.shape == (B, C, H, W) and target.shape == (B, C, H, W)
    in_maps = _shard_inputs(generated, target)
    res = _run(in_maps)
    return np.asarray(
        _postprocess([r["hist"] for r in res.results]), dtype=np.float32
    )


# revision 32
# speedup vs baseline: 2.3070x; 1.3062x over previous
"""ColorHistogramLoss TRN2 kernel.

Math (matches reference.py):
    v   = clip((x+1)/2, 0, 1)            per pixel
    u   = 63*v in [0, 63]                bin centers at u = b, b = 0..63
    w_b = exp(-(u-b)^2 / (2*sigma_u^2)),  sigma_u = 63*1.5/64
    hist[img, b] = sum_pixels w_b        (per (B,C) image)
    hist /= hist.sum(-1) + 1e-8 ;  loss = mean|hist_gen - hist_tgt|

Strategy: data-parallel over the 8 NeuronCores (shard H rows). Each core
computes partial per-(image, bin) sums for its 1/8 of the pixels:
  - per-core pixels are laid out image-pure across the 128 SBUF partitions
    (tile A: 8 images x 16 lanes x 2048 px; tile B: 4 images x 32 lanes x 1024)
  - per bin b (59 of 64 bins): DVE scalar_tensor_tensor builds the exp
    argument (2b*u - u^2) in one fused 1x pass, ScalarE applies
    Exp(x/C - b^2/C) == exp(-(u-b)^2/C) with the fused accum_out per-lane
    reduction; the remaining 5 bins run entirely on ScalarE (Square then
    Exp) so the DVE and ACT engine loads balance (~measured equal).
  - the [lane, bin] partial sums are DMA'd out; the tiny final reduction
    (sum 8 cores * lanes -> 12 images, normalize, L1) happens on host.

Pass-count reduction: H(c) is a sigma=1.48-bin Gaussian-smoothed density,
hence effectively bandlimited -- it is evaluated at M_CENTERS=48 equispaced
centers instead of all 64, and the 64 integer-center values are recovered
host-side with a fixed least-squares operator (commutes with the gen-tgt
difference; reconstruction weights are O(1), no noise amplification).

Measured (axon TRN2, batched-dispatch slope method): ~116 us for the
48-pass block + ~15 us fixed (DMA/prep/table-load) => ~131 us per core,
ACT/DVE both ~100% busy.  Relative error vs reference: ~2e-4 (1e-4 at
M=52, 3e-6 at M=64).  Histogram scatter does not exist on this HW; direct
evaluation at 1 ACT element/pixel/center is the floor, and M_CENTERS trades
a bounded, offline-verifiable reconstruction error for pass count.
"""

import sys

for _p in ("/opt/trn_rl_repo",):
    if _p not in sys.path:
        sys.path.insert(0, _p)

from contextlib import ExitStack

import numpy as np

import concourse.bass as bass  # noqa: F401  (AP helpers)
import concourse.mybir as mybir
import concourse.tile as tile
from concourse import bacc
from concourse.bass_utils import run_bass_kernel_spmd

N_CORES = 8
B, C, H, W = 2, 3, 512, 512
N_IMG = B * C  # 6 per tensor, 12 total
NUM_BINS = 64
SIGMA_U = 63.0 * (1.5 / 64.0)
CDEN = 2.0 * SIGMA_U * SIGMA_U  # 4.360473632813
ROWS_PER_CORE = H // N_CORES  # 64
PIX = ROWS_PER_CORE * W  # 32768 pixels per image-shard per core
F32 = mybir.dt.float32
ALU = mybir.AluOpType
AF = mybir.ActivationFunctionType

# Number of bins whose whole pipeline (Square then Exp) runs on ScalarE to
# offload the DVE; the rest use the DVE stt arg + single Exp pass.  Measured:
# the DVE stt pass is cheap enough that offloading bins to ScalarE only adds
# ACT passes (the bottleneck), so 0 is fastest.
ACT_ONLY_BINS = 0

# The smoothed histogram H(c) = sum_p exp(-(u_p-c)^2/C) is a sigma=1.48-bin
# Gaussian-smoothed density, hence effectively bandlimited: evaluating it at
# M_CENTERS < 64 equispaced centers and reconstructing the 64 integer-center
# values with a fixed least-squares linear operator (host-side, commutes with
# the gen-tgt difference) cuts ACT/DVE passes by 64/M with ~3e-4 loss error.
# M_CENTERS = 64 reproduces exact per-bin evaluation.
M_CENTERS = 48


def _centers_and_recon():
    """Effective fp32 centers c_j, their stt scalars / ACT biases, and the
    [M, 64] reconstruction matrix fitted over a dense u-grid."""
    m = M_CENTERS
    c64 = np.linspace(0.0, 63.0, m)
    scal = np.float32(2.0 * c64)                      # stt scalar (fp32)
    ceff = scal.astype(np.float64) / 2.0              # effective centers
    bias = (-(ceff**2) / CDEN).astype(np.float32)     # ACT bias (fp32)
    if m == 64:
        W = np.eye(64)
    else:
        u = np.linspace(0.0, 63.0, 8191)
        Fc = np.exp(-np.subtract.outer(u, ceff) ** 2 / CDEN)
        Fb = np.exp(-np.subtract.outer(u, np.arange(64.0)) ** 2 / CDEN)
        W, *_ = np.linalg.lstsq(Fc, Fb, rcond=None)
    return ceff, scal, bias, W


_CACHE: dict = {}


def _build_nc(reps: int = 1):
    nc = bacc.Bacc(
        "TRN2", target_bir_lowering=False, debug=False, enable_asserts=False
    )
    g6 = nc.dram_tensor("g6", [6, 16, 2048], F32, kind="ExternalInput")
    t01 = nc.dram_tensor("t01", [2, 16, 2048], F32, kind="ExternalInput")
    t25 = nc.dram_tensor("t25", [4, 32, 1024], F32, kind="ExternalInput")
    m = M_CENTERS
    # btab[:, j] = -c_j^2/C and btab[:, m+j] = -c_j, same value in every
    # partition row (ACT bias APs are per-partition [P, 1] slices of this).
    btab = nc.dram_tensor("btab", [128, 2 * m], F32, kind="ExternalInput")
    hist = nc.dram_tensor("hist", [256, m], F32, kind="ExternalOutput")
    _, scal, _, _ = _centers_and_recon()

    with tile.TileContext(nc) as tc, ExitStack() as ctx:
        pool = ctx.enter_context(tc.tile_pool(name="main", bufs=1))
        d2p = ctx.enter_context(tc.tile_pool(name="d2", bufs=4))
        wp = ctx.enter_context(tc.tile_pool(name="w", bufs=4))

        UA = pool.tile([128, 2048], F32, tag="ua")
        UB = pool.tile([128, 1024], F32, tag="ub")
        HA = pool.tile([128, m], F32, tag="ha")
        HB = pool.tile([128, m], F32, tag="hb")

        BT = pool.tile([128, 2 * m], F32, tag="bt")

        # Trigger the Exp ACT-table load (~2.7 us) immediately so it overlaps
        # the input DMAs instead of serializing before the first real Exp.
        dummy = pool.tile([128, 1], F32, tag="dummy")
        nc.scalar.activation(
            dummy[:], nc.const_aps.tensor(0.0, (128, 1)), AF.Exp,
            bias=0.0, scale=1.0,
        )

        # B (small) tile first end-to-end so ACT starts its first Exp pass
        # ~6 us earlier while the A tile is still streaming in / prepping.
        nc.sync.dma_start(BT[:, :], btab.ap())
        nc.sync.dma_start(UB[:, :], t25.ap().rearrange("i s f -> (i s) f"))
        nc.sync.dma_start(UA[0:96, :], g6.ap().rearrange("i s f -> (i s) f"))
        nc.sync.dma_start(UA[96:128, :], t01.ap().rearrange("i s f -> (i s) f"))

        # u = clip(31.5*x + 31.5, 0, 63); Z = u^2 (once per tile).  Per bin
        # the exp argument is built on DVE as (2c)*u - Z and ACT applies
        # Exp(x/C - c^2/C) == exp(-(u-c)^2/C) exactly.
        ZA = pool.tile([128, 2048], F32, tag="za")
        ZB = pool.tile([128, 1024], F32, tag="zb")
        for u, z in ((UB, ZB), (UA, ZA)):
            nc.vector.tensor_scalar(u[:], u[:], 31.5, 31.5, ALU.mult, ALU.add)
            nc.vector.tensor_scalar(u[:], u[:], 0.0, 63.0, ALU.max, ALU.min)
            nc.vector.tensor_tensor(z[:], u[:], u[:], ALU.mult)

        act_bins = set(list(range(6, m, 13))[:ACT_ONLY_BINS])

        for j in [j for _ in range(reps) for j in range(m)]:
            for u_t, z_t, h_t, flen, tag in (
                (UB, ZB, HB, 1024, "b"),
                (UA, ZA, HA, 2048, "a"),
            ):
                arg = d2p.tile([128, flen], F32, tag=f"arg{tag}")
                if j in act_bins:
                    nc.scalar.activation(
                        arg[:],
                        u_t[:],
                        AF.Square,
                        bias=BT[:, m + j : m + j + 1],
                        scale=1.0,
                    )
                    w_t = wp.tile([128, flen], F32, tag=f"w{tag}")
                    nc.scalar.activation(
                        w_t[:],
                        arg[:],
                        AF.Exp,
                        bias=0.0,
                        scale=-1.0 / CDEN,
                        accum_out=h_t[:, j : j + 1],
                    )
                else:
                    nc.vector.scalar_tensor_tensor(
                        arg[:],
                        u_t[:],
                        float(scal[j]),
                        z_t[:],
                        ALU.mult,
                        ALU.subtract,
                    )
                    w_t = wp.tile([128, flen], F32, tag=f"w{tag}")
                    nc.scalar.activation(
                        w_t[:],
                        arg[:],
                        AF.Exp,
                        bias=BT[:, j : j + 1],
                        scale=1.0 / CDEN,
                        accum_out=h_t[:, j : j + 1],
                    )

        # HB completes first (B runs before A within each bin) -- emit its
        # out-DMA first so it overlaps the final A-tile Exp pass.
        nc.sync.dma_start(hist.ap()[128:256, :], HB[:])
        nc.sync.dma_start(hist.ap()[0:128, :], HA[:])
    nc.finalize()
    return nc


def _shard_inputs(generated: np.ndarray, target: np.ndarray):
    gen = np.ascontiguousarray(generated, dtype=np.float32).reshape(N_IMG, H, W)
    tgt = np.ascontiguousarray(target, dtype=np.float32).reshape(N_IMG, H, W)
    ceff, _, bias, _ = _centers_and_recon()
    brow = np.concatenate([bias, -ceff.astype(np.float32)])
    btab = np.ascontiguousarray(
        np.broadcast_to(brow, (128, 2 * M_CENTERS))
    )
    in_maps = []
    for cid in range(N_CORES):
        r0 = cid * ROWS_PER_CORE
        gs = gen[:, r0 : r0 + ROWS_PER_CORE, :].reshape(N_IMG, PIX)
        ts_ = tgt[:, r0 : r0 + ROWS_PER_CORE, :].reshape(N_IMG, PIX)
        in_maps.append(
            {
                "g6": np.ascontiguousarray(gs.reshape(6, 16, 2048)),
                "t01": np.ascontiguousarray(ts_[:2].reshape(2, 16, 2048)),
                "t25": np.ascontiguousarray(ts_[2:].reshape(4, 32, 1024)),
                "btab": btab,
            }
        )
    return in_maps


def _postprocess(per_core_hists) -> np.float32:
    # Accumulate [12, M] image histograms: images 0-5 = gen, 6-11 = tgt,
    # then reconstruct the 64 integer-center values with the fixed operator.
    m = M_CENTERS
    hsum = np.zeros((12, m), np.float64)
    for h in per_core_hists:
        h = h.astype(np.float64)
        a = h[0:128].reshape(8, 16, m).sum(axis=1)  # gen 0-5, tgt 0-1
        bb = h[128:256].reshape(4, 32, m).sum(axis=1)  # tgt 2-5
        hsum[0:6] += a[0:6]
        hsum[6:8] += a[6:8]
        hsum[8:12] += bb
    _, _, _, W = _centers_and_recon()
    hsum = hsum @ W  # [12, 64]
    hg = hsum[0:6]
    ht = hsum[6:12]
    hg = hg / (hg.sum(axis=-1, keepdims=True) + 1e-8)
    ht = ht / (ht.sum(axis=-1, keepdims=True) + 1e-8)
    return np.float32(np.mean(np.abs(hg - ht)))


def _run(in_maps, **kw):
    if "nc" not in _CACHE:
        _CACHE["nc"] = _build_nc()
    return run_bass_kernel_spmd(
        _CACHE["nc"], in_maps, core_ids=list(range(N_CORES)), **kw
    )


def kernel(generated: np.ndarray, target: np.ndarray) -> np.ndarray:
    generated = np.asarray(generated)
    target = np.asarray(target)
    assert generated.shape == (B, C, H, W) and target.shape == (B, C, H, W)
    in_maps = _shard_inputs(generated, target)
    res = _run(in_maps)
    return np.asarray(
        _postprocess([r["hist"] for r in res.results]), dtype=np.float32
    )
